# revision 1
# baseline (speedup 1.0000x reference)
"""Bilateral filter 7x7 (sigma_space=5, sigma_color=0.1) on Trainium2, 8 cores.

Input  I: (2, 1, 720, 1280) float32, output (2, 720, 1280) float32.

Formulation: out = I + S/W with
  W = 1 + sum_pairs g * (e(p) + e(p - d))
  S = sum_pairs g * (q(p) - q(p - d)),   q = e * d,  e = exp(-50 d^2),
  d_delta(p) = I(p + delta) - I(p)
over the 24 shift pairs delta=(dy,dx), dy>=0 (dy==0 => dx>0); the center tap
contributes g0=1 to W only.  Uses e_{-delta}(p) = e_delta(p-delta) exactly.

Sharding: batch(2) x row-half(2) x col-half(2) -> 8 cores, 360x640 out each.
Layout: packed 3 rows/partition; row shifts = free-dim offsets + a +1
partition shift absorbed into shifted-identity fp16 matmul weights; PE
accumulates the 96 plane sums into PSUM banks.
"""

import math
import numpy as np
from contextlib import ExitStack

import concourse.bass as bass
import concourse.bacc as bacc
import concourse.mybir as mybir
import concourse.tile as tile
from concourse import bass_utils

f32 = mybir.dt.float32
f16 = mybir.dt.float16
AF = mybir.ActivationFunctionType
ALU = mybir.AluOpType

# geometry
HC, WC = 360, 640          # per-core output rows/cols
G = 3                      # rows per partition
PM = HC // G               # 120 matmul out partitions
PK = PM + 1                # 121 rhs partitions
XBS_DEFAULT = (256, 256, 128)  # column block widths (n = 2*XB <= 512)
SLAB_R = 123               # DRAM slab: packed groups (rows -3..365)
SLAB_C = WC + 12           # 652

# pairs grouped by dy
PAIRS = [(0, dx) for dx in (1, 2, 3)] + [
    (dy, dx) for dy in (1, 2, 3) for dx in (-3, -2, -1, 0, 1, 2, 3)
]
GROUPS = [
    [i for i, (dy, _) in enumerate(PAIRS) if dy == d] for d in range(4)
]
NP_ = len(PAIRS)  # 24
SIG_SP, SIG_C = 5.0, 0.1
ESCALE = -1.0 / (2.0 * SIG_C * SIG_C)  # -50


def _gvals():
    return np.array(
        [math.exp(-(dy * dy + dx * dx) / (2.0 * SIG_SP * SIG_SP)) for dy, dx in PAIRS],
        dtype=np.float64,
    )


def _ap(t, offset_elems, dims):
    """Raw AP on tile t: dims = [[step, count], ...]; first dim = partitions."""
    base = t[:]
    return bass.AP(tensor=base.tensor, offset=base.offset + offset_elems, ap=dims)


def build_program(
    sub_eng=("v", "v", "v", "v"),   # per dy-group engine for d-subs: v/g
    sq_eng=("a", "a", "a", "a"),    # per dy-group engine for squares: a/v/g
    qn_eng=("v", "v", "v", "v"),    # per dy-group engine for qn: v/g
    nreps=1,
    timing=False,
    loop=False,
    fin_eng="g",
    xbs=XBS_DEFAULT,
    eq_bufs=3,
    fin_bufs=1,
    d_f16=False,
    loop_unroll=1,
    stages="sqenmf",  # s=subs q=square e=exp n=qn m=matmuls f=finals
):
    nc = bacc.Bacc("TRN2", debug=False)
    if timing:
        probe_d = nc.dram_tensor("probe", (8, 8), f32, kind="ExternalOutput")
    else:
        x_d = nc.dram_tensor("x", (SLAB_R, G, SLAB_C), f32, kind="ExternalInput")
        w_d = nc.dram_tensor("w", (PK, NP_, 2, PM), f16, kind="ExternalInput")
        o_d = nc.dram_tensor("o", (HC, WC), f32, kind="ExternalOutput")

    with tile.TileContext(nc) as tc:
        with ExitStack() as ctx:
            consts = ctx.enter_context(tc.tile_pool(name="consts", bufs=1))
            dpool = ctx.enter_context(tc.tile_pool(name="d", bufs=2))
            Dpool = ctx.enter_context(tc.tile_pool(name="Dsq", bufs=2))
            eqpool = ctx.enter_context(tc.tile_pool(name="eq", bufs=eq_bufs))
            fpool = ctx.enter_context(tc.tile_pool(name="fin", bufs=fin_bufs))
            psum = ctx.enter_context(tc.tile_pool(name="ps", bufs=1, space="PSUM"))

            It = consts.tile([PK + 1, G, SLAB_C], f32)   # rows -3..362
            I1 = consts.tile([PK + 1, G, SLAB_C], f32)   # rows  0..365
            Wt = consts.tile([PK, NP_, 2, PM], f16)
            if timing:
                dram = ctx.enter_context(
                    tc.tile_pool(name="dram", bufs=1, space="DRAM")
                )
                o_d = dram.tile([HC, WC], f32)
                nc.gpsimd.memset(It[:], 0.37)
                nc.gpsimd.memset(I1[:], 0.37)
                nc.gpsimd.memset(Wt[:], 0.01)
            else:
                nc.sync.dma_start(It[:], x_d[0 : PK + 1])
                nc.sync.dma_start(I1[:], x_d[1 : PK + 2])
                nc.sync.dma_start(Wt[:], w_d[:])

            def body(rep):
                xoffs = [sum(xbs[:i]) for i in range(len(xbs))]
                for cb, (x0, XB) in enumerate(zip(xoffs, xbs)):
                    XE = XB + 6
                    A = psum.tile([PM, G, 512], f32, tag="A")
                    B = psum.tile([PM, G, 512], f32, tag="B")
                    region_open = {}

                    def mm(bank, g, lhsT, rhs, last):
                        key = (id(bank), g)
                        st = key not in region_open
                        region_open[key] = True
                        out_ap = _ap(
                            bank, g * 512,
                            [[bank.ap[0][0], PM], [256, 2], [1, XB]],
                        )
                        nc.tensor.matmul(
                            out_ap, lhsT, rhs,
                            start=st, stop=last, skip_group_check=True,
                        )

                    for gi, pair_idx in enumerate(GROUPS):
                        Pk = len(pair_idx)
                        dy = PAIRS[pair_idx[0]][0]
                        dxs = [PAIRS[i][1] for i in pair_idx]
                        dt_ = Dt = eq = None
                        if set(stages) & set("sqn"):
                            dt_ = dpool.tile([PK, Pk, G, XE],
                                             f16 if d_f16 else f32,
                                             tag="d", name="dt_")
                        if set(stages) & set("qe"):
                            Dt = Dpool.tile([PK, Pk, G, XE], f32, tag="D", name="Dt")
                        if set(stages) & set("enm"):
                            eq = eqpool.tile([PK, Pk, G, 2, XE], f16, tag="eq", name="eq")
                            if "e" not in stages:
                                nc.gpsimd.memset(eq[:], 0.25)

                        # --- d subs -------------------------------------
                        # d[p, j, g, x] = I(3p+g-3+dy, x0-3+x+dx_j)
                        #               - I(3p+g-3,    x0-3+x)
                        # I_t[p, g, c]: row 3p+g-3, col c-6 -> c = x+x0+3(+dx)
                        sube = nc.vector if sub_eng[gi] == "v" else nc.gpsimd
                        pstep = G * SLAB_C
                        co = x0 + 3
                        splits = []  # (g_out_start, g_cnt, tile, g_in_start)
                        if dy == 0:
                            splits = [(0, G, It, 0)]
                        elif dy == 1:
                            splits = [(0, 2, It, 1), (2, 1, I1, 0)]
                        elif dy == 2:
                            splits = [(0, 1, It, 2), (1, 2, I1, 0)]
                        else:
                            splits = [(0, G, I1, 0)]
                        for g0_, gcnt, src, gin in (splits if "s" in stages else []):
                            in0 = _ap(
                                src,
                                gin * SLAB_C + co + dxs[0],
                                [[pstep, PK], [1, Pk], [SLAB_C, gcnt], [1, XE]],
                            )
                            in1 = _ap(
                                It,
                                g0_ * SLAB_C + co,
                                [[pstep, PK], [0, Pk], [SLAB_C, gcnt], [1, XE]],
                            )
                            sube.tensor_tensor(
                                out=dt_[:, :, g0_ : g0_ + gcnt, :],
                                in0=in0, in1=in1, op=ALU.subtract,
                            )

                        # --- square ------------------------------------
                        if "q" not in stages:
                            pass
                        elif sq_eng[gi] == "a":
                            nc.scalar.activation(Dt[:], dt_[:], AF.Square)
                        else:
                            sqe = nc.vector if sq_eng[gi] == "v" else nc.gpsimd
                            sqe.tensor_tensor(
                                out=Dt[:], in0=dt_[:], in1=dt_[:], op=ALU.mult
                            )

                        # --- exp ---------------------------------------
                        if "e" in stages:
                            nc.scalar.activation(
                                eq[:, :, :, 0, :], Dt[:], AF.Exp, bias=0.0,
                                scale=ESCALE,
                            )

                        # --- q = +e*d ----------------------------------
                        if "n" in stages:
                            qe = nc.vector if qn_eng[gi] == "v" else nc.gpsimd
                            qe.tensor_tensor(
                                out=eq[:, :, :, 1, :], in0=dt_[:],
                                in1=eq[:, :, :, 0, :], op=ALU.mult,
                            )

                        # --- PE accumulation ---------------------------
                        last_gi = gi == len(GROUPS) - 1
                        for jj, pi in (list(enumerate(pair_idx)) if "m" in stages else []):
                            dx = PAIRS[pi][1]
                            w_up = Wt[:, pi, 0, :]
                            w_id = Wt[0:PM, pi, 1, :]
                            last_pair = last_gi and jj == len(pair_idx) - 1

                            def rhs_view(kparts, jju, gg, xoff):
                                # (j, x) window: j-outer planes, step XE
                                return _ap(
                                    eq,
                                    ((jju * G + gg) * 2) * XE + xoff,
                                    [[eq.ap[0][0], kparts], [XE, 2], [1, XB]],
                                )

                            # term1: rows 3m+g <- eq[m+1, g], cols x (xoff 3)
                            for g in range(G):
                                mm(A, g, w_up,
                                   rhs_view(PK, jj, g, 3), last_pair)
                            # term2: rows 3m+g-dy, cols x-dx
                            for g in range(G):
                                g2 = (g - dy) % 3
                                if g >= dy:
                                    mm(B, g, w_up,
                                       rhs_view(PK, jj, g2, 3 - dx), last_pair)
                                else:
                                    mm(B, g, w_id,
                                       rhs_view(PM, jj, g2, 3 - dx), last_pair)

                    # --- finals ---------------------------------------
                    if "f" not in stages:
                        continue
                    Aw_w, Aw_s = A[:, :, 0:XB], A[:, :, 256 : 256 + XB]
                    Bw_w, Bw_s = B[:, :, 0:XB], B[:, :, 256 : 256 + XB]
                    Wh = fpool.tile([PM, G, XB], f32, tag="Wh")
                    Sh = fpool.tile([PM, G, XB], f32, tag="Sh")
                    Wf = fpool.tile([PM, G, XB], f32, tag="Wf")
                    St = fpool.tile([PM, G, XB], f32, tag="St")
                    Rt = fpool.tile([PM, G, XB], f32, tag="Rt")
                    Ct = fpool.tile([PM, G, XB], f32, tag="Ct")
                    Ot = fpool.tile([PM, G, XB], f32, tag="Ot")
                    # ACT: PSUM->SBUF with +1 / -1 folded (one PSUM input each)
                    nc.scalar.activation(
                        Wh[:], Aw_w, AF.Identity, bias=1.0, scale=1.0
                    )
                    nc.scalar.activation(
                        Sh[:], Aw_s, AF.Copy, bias=0.0, scale=1.0
                    )
                    nc.vector.tensor_tensor(
                        out=Wf[:], in0=Wh[:], in1=Bw_w, op=ALU.add
                    )
                    nc.vector.tensor_tensor(
                        out=St[:], in0=Sh[:], in1=Bw_s, op=ALU.subtract
                    )
                    nc.vector.reciprocal_approx_fast(Rt[:], Wf[:])
                    nc.vector.tensor_tensor(
                        out=Ct[:], in0=St[:], in1=Rt[:], op=ALU.mult
                    )
                    fe = nc.gpsimd if fin_eng == "g" else nc.vector
                    fe.tensor_tensor(
                        out=Ot[:],
                        in0=Ct[:],
                        in1=I1[0:PM, :, x0 + 6 : x0 + 6 + XB],
                        op=ALU.add,
                    )
                    od = o_d[:, x0 : x0 + XB].rearrange(
                        "(p g) x -> p g x", g=G
                    )
                    nc.sync.dma_start(od, Ot[:])

            if loop and nreps > 1:
                assert nreps % loop_unroll == 0
                with tc.For_i(0, nreps // loop_unroll, 1) as _i:
                    for u in range(loop_unroll):
                        body(u)
            else:
                for rep in range(nreps):
                    body(rep)
            if timing:
                nc.sync.dma_start(probe_d[:], o_d[0:8, 0:8])

    nc.compile()
    return nc


def make_inputs(I):
    """I: (2, 1, 720, 1280) float32 -> list of 8 in_maps + weight array."""
    img = np.asarray(I).reshape(2, 720, 1280).astype(np.float32)
    pad = np.pad(img, ((0, 0), (6, 6), (6, 6)))

    g = _gvals()
    w = np.zeros((PK, NP_, 2, PM), np.float16)
    for pi in range(NP_):
        for m in range(PM):
            w[m + 1, pi, 0, m] = g[pi]
            w[m, pi, 1, m] = g[pi]

    in_maps = []
    for b in range(2):
        for r in range(2):
            for wc in range(2):
                r0, c0 = r * HC, wc * WC
                # rows r0-3 .. r0+365, cols c0-6 .. c0+645 (pad-array offsets +6)
                slab = pad[b, r0 + 3 : r0 + 372, c0 : c0 + SLAB_C]
                slab = np.ascontiguousarray(slab).reshape(SLAB_R, G, SLAB_C)
                in_maps.append({"x": slab, "w": w})
    return in_maps


def assemble(results):
    out = np.empty((2, 720, 1280), np.float32)
    i = 0
    for b in range(2):
        for r in range(2):
            for wc in range(2):
                out[b, r * HC : (r + 1) * HC, wc * WC : (wc + 1) * WC] = (
                    results[i]["o"]
                )
                i += 1
    return out


_cached = {}


def kernel(I):
    key = "prog"
    if key not in _cached:
        _cached[key] = build_program()
    nc = _cached[key]
    in_maps = make_inputs(I)
    res = bass_utils.run_bass_kernel_spmd(nc, in_maps, core_ids=list(range(8)))
    return assemble(res.results)



# revision 3
# speedup vs baseline: 12.6302x; 12.6302x over previous
"""Bilateral filter 7x7 (sigma_space=5, sigma_color=0.1) on Trainium2, 8 cores.

Input  I: (2, 1, 720, 1280) float32, output (2, 720, 1280) float32.

Formulation: out = I + S/W with
  W = 1 + sum_pairs g * (e(p) + e(p - d))
  S = sum_pairs g * (q(p) - q(p - d)),   q = e * d,  e = exp(-50 d^2),
  d_delta(p) = I(p + delta) - I(p)
over the 24 shift pairs delta=(dy,dx), dy>=0 (dy==0 => dx>0); the center tap
contributes g0=1 to W only.  Uses e_{-delta}(p) = e_delta(p-delta) exactly.

Sharding: batch(2) x row-half(2) x col-half(2) -> 8 cores, 360x640 out each.
Layout: packed 3 rows/partition; row shifts = free-dim offsets + a +1
partition shift absorbed into shifted-identity fp16 matmul weights; PE
accumulates the plane sums into PSUM banks.

fp16 elementwise chain: the input slab is fp16 and kept in SBUF twice per
row-alignment (even/odd column parity copies) so every dx-shifted read is
4B-aligned, which keeps the DVE tensor_tensor ops in the packed 2x mode.
Pairs within each dy group are ordered even-dx-first so one instruction per
parity class covers them with a step-2 pair dim.
"""

import math
import numpy as np
from contextlib import ExitStack

import concourse.bass as bass
import concourse.bacc as bacc
import concourse.mybir as mybir
import concourse.tile as tile
from concourse import bass_utils

f32 = mybir.dt.float32
f16 = mybir.dt.float16
AF = mybir.ActivationFunctionType
ALU = mybir.AluOpType

# geometry
HC, WC = 360, 640          # per-core output rows/cols
G = 3                      # rows per partition
PM = HC // G               # 120 matmul out partitions
PK = PM + 1                # 121 rhs partitions
XBS_DEFAULT = (256, 256, 128)  # column block widths (n = 2*XB <= 512)
SLAB_R = 123               # DRAM slab: packed groups (rows -3..365)
SLAB_C = WC + 14           # 654 (pad 6 left, 8 right for odd-copy reads)

# pairs grouped by dy; within each group even dx first, then odd dx
# (parity classes let fp16 reads stay 4B-aligned via the parity copies)
PAIRS = [(0, 2), (0, 1), (0, 3)] + [
    (dy, dx) for dy in (1, 2, 3) for dx in (-2, 0, 2, -3, -1, 1, 3)
]
GROUPS = [
    [i for i, (dy, _) in enumerate(PAIRS) if dy == d] for d in range(4)
]
NP_ = len(PAIRS)  # 24
SIG_SP, SIG_C = 5.0, 0.1
ESCALE = -1.0 / (2.0 * SIG_C * SIG_C)  # -50


def _gvals():
    return np.array(
        [math.exp(-(dy * dy + dx * dx) / (2.0 * SIG_SP * SIG_SP)) for dy, dx in PAIRS],
        dtype=np.float64,
    )


def _ap(t, offset_elems, dims):
    """Raw AP on tile t: dims = [[step, count], ...]; first dim = partitions."""
    base = t[:]
    return bass.AP(tensor=base.tensor, offset=base.offset + offset_elems, ap=dims)


def build_program(
    sub_eng=("v", "v", "v", "v"),   # per dy-group engine for d-subs: v/g
    sq_eng=("v", "a", "a", "a"),    # per dy-group engine for squares: a/v/g
    qn_eng=("v", "v", "v", "v"),    # per dy-group engine for qn: v/g
    nreps=1,
    timing=False,
    loop=False,
    fin_eng="g",
    xbs=XBS_DEFAULT,
    eq_bufs=3,
    fin_bufs=1,
    d_f16=True,                     # kept for call compat; chain is fp16
    loop_unroll=1,
    stages="sqenmf",  # s=subs q=square e=exp n=qn m=matmuls f=finals
):
    nc = bacc.Bacc("TRN2", debug=False)
    if timing:
        probe_d = nc.dram_tensor("probe", (8, 8), f32, kind="ExternalOutput")
    else:
        x_d = nc.dram_tensor("x", (SLAB_R, G, SLAB_C), f16, kind="ExternalInput")
        w_d = nc.dram_tensor("w", (PK, NP_, 2, PM), f16, kind="ExternalInput")
        o_d = nc.dram_tensor("o", (HC, WC), f32, kind="ExternalOutput")

    with tile.TileContext(nc) as tc:
        with ExitStack() as ctx:
            consts = ctx.enter_context(tc.tile_pool(name="consts", bufs=1))
            dpool = ctx.enter_context(tc.tile_pool(name="d", bufs=2))
            Dpool = ctx.enter_context(tc.tile_pool(name="Dsq", bufs=2))
            eqpool = ctx.enter_context(tc.tile_pool(name="eq", bufs=eq_bufs))
            fpool = ctx.enter_context(tc.tile_pool(name="fin", bufs=fin_bufs))
            psum = ctx.enter_context(tc.tile_pool(name="ps", bufs=1, space="PSUM"))

            # fp16 input copies: rows -3.. (It*) and rows 0.. (I1*), each at
            # even (as-is) and odd (shifted 1 col) alignment.
            It_e = consts.tile([PK + 1, G, SLAB_C], f16)
            It_o = consts.tile([PK + 1, G, SLAB_C], f16)
            I1_e = consts.tile([PK + 1, G, SLAB_C], f16)
            I1_o = consts.tile([PK + 1, G, SLAB_C], f16)
            Wt = consts.tile([PK, NP_, 2, PM], f16)
            if timing:
                dram = ctx.enter_context(
                    tc.tile_pool(name="dram", bufs=1, space="DRAM")
                )
                o_d = dram.tile([HC, WC], f32)
                nc.gpsimd.memset(It_e[:], 0.37)
                nc.gpsimd.memset(It_o[:], 0.37)
                nc.gpsimd.memset(I1_e[:], 0.37)
                nc.gpsimd.memset(I1_o[:], 0.37)
                nc.gpsimd.memset(Wt[:], 0.01)
            else:
                nc.sync.dma_start(It_e[:], x_d[0 : PK + 1])
                nc.sync.dma_start(I1_e[:], x_d[1 : PK + 2])
                nc.sync.dma_start(
                    It_o[:, :, 0 : SLAB_C - 1], x_d[0 : PK + 1, :, 1:SLAB_C]
                )
                nc.sync.dma_start(
                    I1_o[:, :, 0 : SLAB_C - 1], x_d[1 : PK + 2, :, 1:SLAB_C]
                )
                nc.sync.dma_start(Wt[:], w_d[:])

            def pick(copies, off):
                """Return (tile, element offset) with 4B-aligned offset."""
                ev, od = copies
                if off % 2 == 0:
                    return ev, off
                return od, off - 1

            def body(rep):
                xoffs = [sum(xbs[:i]) for i in range(len(xbs))]
                for cb, (x0, XB) in enumerate(zip(xoffs, xbs)):
                    XE = XB + 6
                    A = psum.tile([PM, G, 512], f32, tag="A")
                    B = psum.tile([PM, G, 512], f32, tag="B")
                    region_open = {}

                    def mm(bank, g, lhsT, rhs, last):
                        key = (id(bank), g)
                        st = key not in region_open
                        region_open[key] = True
                        out_ap = _ap(
                            bank, g * 512,
                            [[bank.ap[0][0], PM], [256, 2], [1, XB]],
                        )
                        nc.tensor.matmul(
                            out_ap, lhsT, rhs,
                            start=st, stop=last, skip_group_check=True,
                        )

                    for gi, pair_idx in enumerate(GROUPS):
                        Pk = len(pair_idx)
                        dy = PAIRS[pair_idx[0]][0]
                        dxs = [PAIRS[i][1] for i in pair_idx]
                        dt_ = Dt = eq = None
                        if set(stages) & set("sqn"):
                            dt_ = dpool.tile([PK, Pk, G, XE], f16, tag="d", name="dt_")
                        if set(stages) & set("qe"):
                            Dt = Dpool.tile([PK, Pk, G, XE], f16, tag="D", name="Dt")
                        if set(stages) & set("enm"):
                            eq = eqpool.tile([PK, Pk, G, 2, XE], f16, tag="eq", name="eq")
                            if "e" not in stages:
                                nc.gpsimd.memset(eq[:], 0.25)

                        # --- d subs -------------------------------------
                        # d[p, j, g, x] = I(3p+g-3+dy, x0-3+x+dx_j)
                        #               - I(3p+g-3,    x0-3+x)
                        # I tiles: row 3p+g-3, col c-6 -> c = x+x0+3(+dx)
                        sube = nc.vector if sub_eng[gi] == "v" else nc.gpsimd
                        pstep = G * SLAB_C
                        co = x0 + 3
                        splits = []  # (g_out_start, g_cnt, copies, g_in_start)
                        if dy == 0:
                            splits = [(0, G, (It_e, It_o), 0)]
                        elif dy == 1:
                            splits = [(0, 2, (It_e, It_o), 1), (2, 1, (I1_e, I1_o), 0)]
                        elif dy == 2:
                            splits = [(0, 1, (It_e, It_o), 2), (1, 2, (I1_e, I1_o), 0)]
                        else:
                            splits = [(0, G, (I1_e, I1_o), 0)]
                        # parity classes: pairs [0:neven] have even dx,
                        # [neven:Pk] odd dx, each ascending with step 2
                        neven = sum(1 for dx in dxs if dx % 2 == 0)
                        classes = [(0, neven), (neven, Pk - neven)]
                        in1_t, in1_off = pick((It_e, It_o), co)
                        for g0_, gcnt, copies, gin in (splits if "s" in stages else []):
                            for j0, jn in classes:
                                if jn == 0:
                                    continue
                                base_t, base_off = pick(
                                    copies, gin * SLAB_C + co + dxs[j0]
                                )
                                in0 = _ap(
                                    base_t,
                                    base_off,
                                    [[pstep, PK], [2, jn], [SLAB_C, gcnt], [1, XE]],
                                )
                                in1 = _ap(
                                    in1_t,
                                    g0_ * SLAB_C + in1_off,
                                    [[pstep, PK], [0, jn], [SLAB_C, gcnt], [1, XE]],
                                )
                                sube.tensor_tensor(
                                    out=dt_[:, j0 : j0 + jn, g0_ : g0_ + gcnt, :],
                                    in0=in0, in1=in1, op=ALU.subtract,
                                )

                        # --- square ------------------------------------
                        if "q" not in stages:
                            pass
                        elif sq_eng[gi] == "a":
                            nc.scalar.activation(Dt[:], dt_[:], AF.Square)
                        else:
                            sqe = nc.vector if sq_eng[gi] == "v" else nc.gpsimd
                            sqe.tensor_tensor(
                                out=Dt[:], in0=dt_[:], in1=dt_[:], op=ALU.mult
                            )

                        # --- exp ---------------------------------------
                        if "e" in stages:
                            nc.scalar.activation(
                                eq[:, :, :, 0, :], Dt[:], AF.Exp, bias=0.0,
                                scale=ESCALE,
                            )

                        # --- q = +e*d ----------------------------------
                        if "n" in stages:
                            qe = nc.vector if qn_eng[gi] == "v" else nc.gpsimd
                            qe.tensor_tensor(
                                out=eq[:, :, :, 1, :], in0=dt_[:],
                                in1=eq[:, :, :, 0, :], op=ALU.mult,
                            )

                        # --- PE accumulation ---------------------------
                        last_gi = gi == len(GROUPS) - 1
                        for jj, pi in (list(enumerate(pair_idx)) if "m" in stages else []):
                            dx = PAIRS[pi][1]
                            w_up = Wt[:, pi, 0, :]
                            w_id = Wt[0:PM, pi, 1, :]
                            last_pair = last_gi and jj == len(pair_idx) - 1

                            def rhs_view(kparts, jju, gg, xoff):
                                # (j, x) window: j-outer planes, step XE
                                return _ap(
                                    eq,
                                    ((jju * G + gg) * 2) * XE + xoff,
                                    [[eq.ap[0][0], kparts], [XE, 2], [1, XB]],
                                )

                            # term1: rows 3m+g <- eq[m+1, g], cols x (xoff 3)
                            for g in range(G):
                                mm(A, g, w_up,
                                   rhs_view(PK, jj, g, 3), last_pair)
                            # term2: rows 3m+g-dy, cols x-dx
                            for g in range(G):
                                g2 = (g - dy) % 3
                                if g >= dy:
                                    mm(B, g, w_up,
                                       rhs_view(PK, jj, g2, 3 - dx), last_pair)
                                else:
                                    mm(B, g, w_id,
                                       rhs_view(PM, jj, g2, 3 - dx), last_pair)

                    # --- finals ---------------------------------------
                    if "f" not in stages:
                        continue
                    Aw_w, Aw_s = A[:, :, 0:XB], A[:, :, 256 : 256 + XB]
                    Bw_w, Bw_s = B[:, :, 0:XB], B[:, :, 256 : 256 + XB]
                    Wh = fpool.tile([PM, G, XB], f32, tag="Wh")
                    Sh = fpool.tile([PM, G, XB], f32, tag="Sh")
                    Wf = fpool.tile([PM, G, XB], f32, tag="Wf")
                    St = fpool.tile([PM, G, XB], f32, tag="St")
                    Rt = fpool.tile([PM, G, XB], f32, tag="Rt")
                    Ct = fpool.tile([PM, G, XB], f32, tag="Ct")
                    Ot = fpool.tile([PM, G, XB], f32, tag="Ot")
                    # ACT: PSUM->SBUF with +1 / -1 folded (one PSUM input each)
                    nc.scalar.activation(
                        Wh[:], Aw_w, AF.Identity, bias=1.0, scale=1.0
                    )
                    nc.scalar.activation(
                        Sh[:], Aw_s, AF.Copy, bias=0.0, scale=1.0
                    )
                    nc.vector.tensor_tensor(
                        out=Wf[:], in0=Wh[:], in1=Bw_w, op=ALU.add
                    )
                    nc.vector.tensor_tensor(
                        out=St[:], in0=Sh[:], in1=Bw_s, op=ALU.subtract
                    )
                    nc.vector.reciprocal_approx_fast(Rt[:], Wf[:])
                    nc.vector.tensor_tensor(
                        out=Ct[:], in0=St[:], in1=Rt[:], op=ALU.mult
                    )
                    fe = nc.gpsimd if fin_eng == "g" else nc.vector
                    fe.tensor_tensor(
                        out=Ot[:],
                        in0=Ct[:],
                        in1=I1_e[0:PM, :, x0 + 6 : x0 + 6 + XB],
                        op=ALU.add,
                    )
                    od = o_d[:, x0 : x0 + XB].rearrange(
                        "(p g) x -> p g x", g=G
                    )
                    nc.sync.dma_start(od, Ot[:])

            if loop and nreps > 1:
                assert nreps % loop_unroll == 0
                with tc.For_i(0, nreps // loop_unroll, 1) as _i:
                    for u in range(loop_unroll):
                        body(u)
            else:
                for rep in range(nreps):
                    body(rep)
            if timing:
                nc.sync.dma_start(probe_d[:], o_d[0:8, 0:8])

    nc.compile()
    return nc


def make_inputs(I):
    """I: (2, 1, 720, 1280) float32 -> list of 8 in_maps + weight array."""
    img = np.asarray(I).reshape(2, 720, 1280).astype(np.float32)
    pad = np.pad(img, ((0, 0), (6, 6), (6, 8)))

    g = _gvals()
    w = np.zeros((PK, NP_, 2, PM), np.float16)
    for pi in range(NP_):
        for m in range(PM):
            w[m + 1, pi, 0, m] = g[pi]
            w[m, pi, 1, m] = g[pi]

    in_maps = []
    for b in range(2):
        for r in range(2):
            for wc in range(2):
                r0, c0 = r * HC, wc * WC
                # rows r0-3 .. r0+365, cols c0-6 .. c0+647 (pad offsets +6)
                slab = pad[b, r0 + 3 : r0 + 372, c0 : c0 + SLAB_C]
                slab = (
                    np.ascontiguousarray(slab)
                    .reshape(SLAB_R, G, SLAB_C)
                    .astype(np.float16)
                )
                in_maps.append({"x": slab, "w": w})
    return in_maps


def assemble(results):
    out = np.empty((2, 720, 1280), np.float32)
    i = 0
    for b in range(2):
        for r in range(2):
            for wc in range(2):
                out[b, r * HC : (r + 1) * HC, wc * WC : (wc + 1) * WC] = (
                    results[i]["o"]
                )
                i += 1
    return out


_cached = {}


def kernel(I):
    key = "prog"
    if key not in _cached:
        _cached[key] = build_program()
    nc = _cached[key]
    in_maps = make_inputs(I)
    res = bass_utils.run_bass_kernel_spmd(nc, in_maps, core_ids=list(range(8)))
    return assemble(res.results)
